# revision 1
# baseline (speedup 1.0000x reference)
"""Trainium2 Bass kernel for column-softmax attention.

reference semantics:
    scores = einsum('bqd,bkd->bqk', q, k) / sqrt(128)   # [B, Nq, Nk]
    attn   = softmax(scores, axis=1)                     # over the QUERY axis
    out    = einsum('bqk,bkd->bqd', attn, v)             # [B, Nq, D]

Because the softmax is over q, each key column normalizes independently:
    out[q, d] = sum_k E[k, q] * r[k] * v[k, d],  E = exp(scores.T), r = 1/sum_q E[k, q]

Sharding: 8 cores = 4 batches x 2 key-halves.  Each core computes the partial
sum over its 2048 keys; the host adds the two partials per batch.

The ScalarE exp pass (8.4M elements/core) is the roofline (~55us of pure
element cycles + per-instruction overhead); the design goal is zero ACT
idle with the widest ACT reads the 8 PSUM banks allow:

- Single pipelined loop over 16 key tiles.  Per key tile: scores matmuls
  (fp16, 512-col chunks) into a double-buffered [128,1536] PSUM tile
  (6 banks); exp on ScalarE in 1536/1536/1024-wide reads with row-sums
  fused via accum_out.
- The AV GEMM (out.T[d,q] += vsc^T E) is *streamed*: key tiles are grouped
  in pairs; a pair's AV contribution is computed during the NEXT pair's exp
  window as 512-col chunks in the 2 remaining PSUM banks, each immediately
  flush-added into an SBUF fp32 accumulator by VectorE (GPSIMD cannot read
  PSUM).  This removes the baseline's 23us ACT-idle tail (phase-B AV pass).
  Chunk pairs share one stationary (vsc) load per key tile to halve PE
  weight loads.
- Input tiles are double-buffered so the next loop iteration's DMAs
  prefetch during this iteration; DMAs are ordered by first use (HWDGE
  descriptor generation is serialized, ~625ns each).
- The last group's flush-adds write an fp16 staging buffer (halves output
  DMA bytes); output chunks DMA out as soon as their final add lands.
- An exp at t=0 (outside the timing loop) preloads the ACT table set.
"""

import numpy as np

import concourse.bass as bass
import concourse.mybir as mybir
import concourse.tile as tile
from concourse.bass_utils import run_bass_kernel_spmd

B, N, D = 4, 4096, 128
P = 128
NK = 2048                 # keys per core (half of 4096)
KT_TILES = NK // P        # 16 key tiles of 128
SCALE = 1.0 / np.sqrt(128.0)

F32 = mybir.dt.float32
F16 = mybir.dt.float16

# AV interleave groups: a group's AV matmuls run during the next group's
# exp window; the final [15] group is the (short) tail.
GROUPS_G2 = [[0], [1, 2], [3, 4], [5, 6], [7, 8], [9, 10], [11, 12], [13, 14], [15]]
GROUPS_G3 = [[0], [1, 2, 3], [4, 5, 6], [7, 8, 9], [10, 11, 12], [13, 14], [15]]
N_CHUNK = N // 512        # 8 AV chunks of 512 queries


def emit_body(nc, tc, pools, aps, act_w=1024, g3=False, dve_accum=0):
    big, inp, epool, small, spsum, opsum = pools
    qt_d, kt_d, v_d, out_d = aps
    groups = GROUPS_G3 if g3 else GROUPS_G2
    if act_w == 2048:
        chunks = [(0, 2048), (2048, 4096)]
        drain_hi = None  # AV rides the S-pool rotation, 1 tile per key tile
    elif act_w == 1536:
        chunks = [(0, 1536), (1536, 3072), (3072, 4096)]
        drain_hi = 99 if g3 else 6
    else:
        chunks = [(h * 1024, (h + 1) * 1024) for h in range(4)]
        drain_hi = 4

    # Input tiles double-buffered (bufs=2) so the NEXT loop iteration's input
    # DMAs prefetch during this iteration instead of stalling at the boundary.
    qT = inp.tile([P, N], F16, tag="qT")            # [d, q]
    kT = inp.tile([P, NK], F16, tag="kT")           # [d, k]
    vsb = inp.tile([P, KT_TILES, D], F16, tag="v")  # [k_in_tile, k_tile, d]
    oacc = big.tile([P, N], F32, tag="oacc")        # [d, q] SBUF accumulator
    obuf = big.tile([P, N], F16, tag="obuf")        # final sums, fp16 for DMA

    # Input DMAs on one queue, ordered by first use (the DMA engines and the
    # HWDGE descriptor generator are serialized devices; an early bulk DMA
    # would delay the critical first q/k chunks).
    v_r = v_d.rearrange("(t p) d -> p t d", p=P)
    nc.sync.dma_start(kT[:, 0:P], kt_d[:, 0:P])
    nc.sync.dma_start(qT[:, 0:512], qt_d[:, 0:512])
    nc.sync.dma_start(qT[:, 512:2048], qt_d[:, 512:2048])
    nc.sync.dma_start(qT[:, 2048:4096], qt_d[:, 2048:4096])
    nc.sync.dma_start(vsb[:, 0:1, :], v_r[:, 0:1, :])
    nc.sync.dma_start(kT[:, P:NK], kt_d[:, P:NK])
    nc.sync.dma_start(vsb[:, 1:16, :], v_r[:, 1:16, :])

    # Warm-up matmul: first real matmul then carries at most one sync wait.
    # Depends only on the (tiny) kT tile-0 DMA.
    Swarm = spsum.tile([P, act_w], F32, tag="S")
    nc.tensor.matmul(
        Swarm[0:1, 0:1], lhsT=kT[:, 0:1], rhs=kT[:, 0:1], start=True, stop=True
    )

    e_tiles = {}
    vsc_tiles = {}
    group_of = {}
    for gi, g in enumerate(groups):
        for kt in g:
            group_of[kt] = gi

    av_queue = []       # pending (gi, chunk) AV blocks
    n_flush = [0]

    def emit_av_blocks(n):
        """Emit n pending AV chunk blocks.  When emitting 2, interleave the
        two chunks' matmuls per stationary operand so each vsc weight is
        loaded once for both chunks (halves PE weight loads)."""
        take = []
        while n > 0 and av_queue:
            take.append(av_queue.pop(0))
            n -= 1
        if not take:
            return
        gi = take[0][0]
        if any(g != gi for g, _ in take):
            # split at a group boundary: emit each run separately
            for g, c in reversed(take):
                av_queue.insert(0, (g, c))
            run = [av_queue.pop(0)]
            while av_queue and av_queue[0][0] == run[0][0] and len(run) < len(take):
                run.append(av_queue.pop(0))
            rest = len(take) - len(run)
            take = run
            gi = take[0][0]
        else:
            rest = 0
        g = groups[gi]
        ots = []
        for _t in take:
            Ot_i = opsum.tile([P, 512], F32, tag="O")
            ots.append(Ot_i)
        for j, ktg in enumerate(g):
            for (_, c), Ot in zip(take, ots):
                nc.tensor.matmul(
                    Ot[:],
                    lhsT=vsc_tiles[ktg][:],
                    rhs=e_tiles[ktg][:, c * 512 : (c + 1) * 512],
                    start=(j == 0),
                    stop=(j == len(g) - 1),
                )
        for (_, c), Ot in zip(take, ots):
            lo = c * 512
            # GPSIMD cannot read PSUM; all flushes go through VectorE.
            eng = nc.vector
            n_flush[0] += 1
            if gi == 0:
                eng.tensor_copy(out=oacc[:, lo : lo + 512], in_=Ot[:])
            elif gi == len(groups) - 1:
                # Final contribution: add into the fp16 DMA staging buffer
                # (halves output-DMA bytes in the tail).
                eng.tensor_add(obuf[:, lo : lo + 512], Ot[:], oacc[:, lo : lo + 512])
                if c % 2 == 1:
                    dlo = (c - 1) * 512
                    nc.sync.dma_start(
                        out_d[:, dlo : dlo + 1024], obuf[:, dlo : dlo + 1024]
                    )
            else:
                eng.tensor_add(oacc[:, lo : lo + 512], Ot[:], oacc[:, lo : lo + 512])
        if rest:
            emit_av_blocks(rest)

    def emit_av_tile_2048():
        """act_w=2048 mode: one [128,2048] AV tile allocated from the S pool
        itself (all 8 PSUM banks belong to it); 4 chunk-columns of 512, each
        accumulated over the group, then one wide flush-add."""
        gi, half = av_queue.pop(0)
        g = groups[gi]
        Ot = spsum.tile([P, 2048], F32, tag="S")
        for j, ktg in enumerate(g):
            for c in range(4):
                nc.tensor.matmul(
                    Ot[:, c * 512 : (c + 1) * 512],
                    lhsT=vsc_tiles[ktg][:],
                    rhs=e_tiles[ktg][:, half * 2048 + c * 512 : half * 2048 + (c + 1) * 512],
                    start=(j == 0),
                    stop=(j == len(g) - 1),
                )
        lo = half * 2048
        if gi == 0:
            nc.vector.tensor_copy(out=oacc[:, lo : lo + 2048], in_=Ot[:])
        elif gi == len(groups) - 1:
            nc.vector.tensor_add(obuf[:, lo : lo + 2048], Ot[:], oacc[:, lo : lo + 2048])
            nc.sync.dma_start(out_d[:, lo : lo + 2048], obuf[:, lo : lo + 2048])
        else:
            nc.vector.tensor_add(oacc[:, lo : lo + 2048], Ot[:], oacc[:, lo : lo + 2048])

    for kt in range(KT_TILES):
        E = epool.tile([P, N], F16, tag=f"E{kt}")   # [k, q] = exp(scores.T)
        rs = small.tile([P, len(chunks)], F32, tag="rs")
        for h, (lo_q, hi_q) in enumerate(chunks):
            S = spsum.tile([P, act_w], F32, tag="S")
            w = hi_q - lo_q
            for u in range(w // 512):
                nc.tensor.matmul(
                    S[:, u * 512 : (u + 1) * 512],
                    lhsT=kT[:, kt * P : (kt + 1) * P],
                    rhs=qT[:, lo_q + u * 512 : lo_q + u * 512 + 512],
                    start=True,
                    stop=True,
                )
            if h >= len(chunks) - dve_accum:
                # Row-sum on VectorE (from the fp16 E tile, 2x-rate) instead
                # of ACT's accum_out: saves the ~300-500ns accumulator-read
                # per activation at the cost of idle DVE cycles.
                nc.scalar.activation(
                    out=E[:, lo_q:hi_q],
                    in_=S[:, 0:w],
                    func=mybir.ActivationFunctionType.Exp,
                    scale=float(SCALE),
                )
                nc.vector.reduce_sum(
                    out=rs[:, h : h + 1],
                    in_=E[:, lo_q:hi_q],
                    axis=mybir.AxisListType.X,
                )
            else:
                nc.scalar.activation(
                    out=E[:, lo_q:hi_q],
                    in_=S[:, 0:w],
                    func=mybir.ActivationFunctionType.Exp,
                    scale=float(SCALE),
                    accum_out=rs[:, h : h + 1],
                )
            if act_w == 2048:
                # one AV tile per key tile, after the SECOND activation so
                # the S-pool rotation stays [Sa, Sb, AV] (AV reuses Sa's
                # banks, which ACT has finished reading by then).
                if h == 1 and av_queue:
                    emit_av_tile_2048()
            else:
                n_emit = 2 if len(av_queue) >= drain_hi else (1 if av_queue else 0)
                emit_av_blocks(n_emit)
        rsum = small.tile([P, 1], F32, tag="rsum")
        recip = small.tile([P, 1], F32, tag="recip")
        vsc = small.tile([P, D], F16, tag=f"vsc{kt}")
        nc.vector.reduce_sum(out=rsum[:], in_=rs[:], axis=mybir.AxisListType.X)
        nc.vector.reciprocal(recip[:], rsum[:])
        nc.vector.tensor_scalar_mul(vsc[:], vsb[:, kt, :], recip[:])
        e_tiles[kt] = E
        vsc_tiles[kt] = vsc

        gi = group_of[kt]
        if kt == groups[gi][-1]:
            if act_w == 2048:
                av_queue.append((gi, 0))
                av_queue.append((gi, 1))
            else:
                for c in range(N_CHUNK):
                    av_queue.append((gi, c))

    # Tail: drain remaining AV blocks (last pair's leftovers + [15]).
    while av_queue:
        if act_w == 2048:
            emit_av_tile_2048()
        else:
            emit_av_blocks(2)


def build_bass(repeat=1, loop=False, act_w=1536, g3=False, dve_accum=0):
    nc = bass.Bass("TRN2", target_bir_lowering=False, debug=False)
    qt_d = nc.dram_tensor("qt", [P, N], F16, kind="ExternalInput").ap()
    kt_d = nc.dram_tensor("kt", [P, NK], F16, kind="ExternalInput").ap()
    v_d = nc.dram_tensor("v", [NK, D], F16, kind="ExternalInput").ap()
    out_d = nc.dram_tensor("out_t", [P, N], F16, kind="ExternalOutput").ap()

    with tile.TileContext(nc) as tc:
        with (
            tc.tile_pool(name="big", bufs=1) as big,
            tc.tile_pool(name="inp", bufs=2) as inp,
            tc.tile_pool(name="epool", bufs=1) as epool,
            tc.tile_pool(name="small", bufs=2) as small,
            tc.tile_pool(name="spsum", bufs=2, space="PSUM") as spsum,
            tc.tile_pool(
                name="opsum", bufs=(2 if act_w == 1536 else 4), space="PSUM"
            ) as opsum,
        ):
            def body():
                emit_body(nc, tc, (big, inp, epool, small, spsum, opsum),
                          (qt_d, kt_d, v_d, out_d), act_w=act_w, g3=g3,
                          dve_accum=dve_accum)

            # ACT table preload at t=0, outside the loop: overlaps the first
            # iteration's input DMAs and costs later iterations nothing.
            wrm = big.tile([P, 1], F16, tag="wrm")
            wrmo = big.tile([P, 1], F16, tag="wrmo")
            nc.vector.memset(wrm[:], 0.0)
            nc.scalar.activation(out=wrmo[:], in_=wrm[:],
                                 func=mybir.ActivationFunctionType.Exp, scale=1.0)

            if loop and repeat > 1:
                with tc.For_i(
                    0, repeat, 1,
                    hint_engines=(mybir.EngineType.PE, mybir.EngineType.Activation),
                ):
                    body()
            else:
                for _ in range(repeat):
                    body()
    return nc


def legalize_waits(nc, max_waits=1):
    """Hoist excess semaphore waits into standalone EventSemaphore ops.

    The walrus codegen for several engine instruction structs accepts only a
    single sync-wait command; Tile sometimes emits more.  Executing the extra
    waits in a preceding same-engine EventSemaphore is semantically identical
    (the engine runs its stream in order).
    """
    for fn in nc.m.functions:
        for blk in fn.blocks:
            out = []
            for inst in blk.instructions:
                si = inst.sync_info
                if (
                    si is not None
                    and si.on_wait
                    and len(si.on_wait) > max_waits
                    and inst.opcode != "EventSemaphore"
                ):
                    waits = list(si.on_wait)
                    extra, keep = waits[:-max_waits], waits[-max_waits:]
                    for n, w in enumerate(extra):
                        out.append(
                            mybir.InstEventSemaphore(
                                name=f"{inst.name}_prewait{n}",
                                engine=inst.engine,
                                ins=[],
                                outs=[],
                                sync_info=mybir.SyncInfo(on_wait=[w], on_update=[]),
                            )
                        )
                    si.on_wait = keep
                out.append(inst)
            blk.instructions = out
    return nc


_NC_CACHE = {}


def _get_nc(repeat=1, **kw):
    key = ("nc", repeat, tuple(sorted(kw.items())))
    if key not in _NC_CACHE:
        _NC_CACHE[key] = legalize_waits(build_bass(repeat, **kw))
    return _NC_CACHE[key]


def kernel(q, k, v):
    q = np.asarray(q, dtype=np.float32)
    k = np.asarray(k, dtype=np.float32)
    v = np.asarray(v, dtype=np.float32)

    in_maps = []
    for c in range(8):
        b, h = c // 2, c % 2
        in_maps.append(
            {
                "qt": np.ascontiguousarray(q[b].T).astype(np.float16),
                "kt": np.ascontiguousarray(k[b, h * NK : (h + 1) * NK].T).astype(np.float16),
                "v": np.ascontiguousarray(v[b, h * NK : (h + 1) * NK]).astype(np.float16),
            }
        )

    nc = _get_nc()
    res = run_bass_kernel_spmd(nc, in_maps, list(range(8))).results

    out = np.empty((B, N, D), dtype=np.float32)
    for b in range(B):
        out[b] = (
            res[2 * b]["out_t"].astype(np.float32)
            + res[2 * b + 1]["out_t"].astype(np.float32)
        ).T
    return out

